# revision 1
# baseline (speedup 1.0000x reference)
"""EventWarping (contrast-maximization loss) Trainium2 kernel.

Strategy: the bilinear splat of each event is a rank-1 outer product
gy (x) gx of two length-256 indicator vectors (2 adjacent nonzeros
each).  A chunk of 128 events therefore accumulates into the 256x256
per-polarity IWE histograms as one-hot matmuls on the PE:

    H[r, c] += sum_e  gy[e, r] * (var_e * gx[e, c])

with events on the contraction (K=128) dim.  The moving operand is the
polarity-masked column-indicator pair [gx*pol+ | gx*pol-] ([128,512]);
the stationary operands are the row indicators gy and (for the
ts-weighted histograms) gyts = gy*ts, each split into 2 row-halves of
128.  All 8 accumulating histograms (2 warp passes x {iwe, ts_iwe} x
2 row-halves) live in the 8 PSUM banks for the whole kernel.

Sharding: batch b -> cores 4b..4b+3, each core takes 250k of that
batch's 1M events (data-parallel over event chunks, replicated
histograms per shard).  Per-core partial histograms are summed and
the (tiny) normalization/loss reduction computed on the host after
gathering.
"""

import numpy as np

import concourse.bacc as bacc
import concourse.bass as bass
import concourse.mybir as mybir
import concourse.tile as tile
from concourse.bass_utils import run_bass_kernel_spmd

P = 128
HW = 256          # histogram height/width
GS = 32           # chunks per group (one For_i iteration)
NG = 62           # groups per core
NCH = NG * GS     # 1984 chunks/core
EVC = NCH * P     # 253952 padded events per core
NCORES = 8
CORES_PER_BATCH = 4
EV_REAL = 250_000  # real events per core (1M per batch / 4 cores)
FS = 256.0        # flow scaling
EPS = 1e-9

F16 = mybir.dt.float16
F32 = mybir.dt.float32
AF = mybir.ActivationFunctionType
OP = mybir.AluOpType

LAST_EXEC_NS = None
LAST_RESULTS = None


def _bcast(ap_col, n):
    """[128,1] AP -> [128,n] broadcast AP (free-dim step 0)."""
    return bass.AP(ap_col.tensor, ap_col.offset,
                   [list(ap_col.ap[0]), [0, n]])


def build_program(ng=NG, kvar=None, loop_ng=None):
    """Builds the SPMD single-core program (identical on all 8 cores).
    kvar: perf-ablation variant knob (None = production).
    loop_ng: process only the first loop_ng groups (same I/O shapes) —
    used to measure pure loop time by differencing two builds."""
    import os
    kvar = kvar or os.environ.get("KVAR", "tsw")
    if loop_ng is None:
        loop_ng = ng
    nc = bacc.Bacc("TRN2", target_bir_lowering=False, debug=False,
                   num_devices=NCORES)

    fields = nc.dram_tensor("fields", [P, ng * 6 * GS], F32,
                            kind="ExternalInput")
    iotas = nc.dram_tensor("iotas", [P, 3 * HW], F16, kind="ExternalInput")
    hist = nc.dram_tensor("hist", [8, P, 512], F32, kind="ExternalOutput")

    with tile.TileContext(nc) as tc:
        with (
            tc.tile_pool(name="const", bufs=1) as constp,
            tc.tile_pool(name="stage", bufs=2) as stagep,
            tc.tile_pool(name="drv", bufs=2) as drvp,
            tc.tile_pool(name="oh", bufs=int(os.environ.get("KBUFS", "3"))) as ohp,
            tc.tile_pool(name="rhs", bufs=int(os.environ.get("KBUFS", "3"))) as rhsp,
            tc.tile_pool(name="psum", bufs=1, space="PSUM") as psump,
            tc.tile_pool(name="out", bufs=1) as outp,
        ):
            iot = constp.tile([P, 3 * HW], F16)
            nc.sync.dma_start(iot[:], iotas.ap())
            iota_p1 = iot[:, 0:HW]          # c + 1
            niota_p1 = iot[:, HW:2 * HW]    # 1 - c
            iota_c = iot[:, 2 * HW:3 * HW]  # c

            zl = constp.tile([P, P], F16)
            nc.vector.memset(zl[:], 0.0)
            zr = constp.tile([P, 512], F16)
            nc.vector.memset(zr[:], 0.0)

            # 8 accumulator banks: [pass(2) x half(2) x vpair(2)] x [128,512]
            banks = [psump.tile([P, 512], F32, tag=f"bank{i}",
                                name=f"bank{i}")
                     for i in range(8)]
            # open accumulation groups + zero
            for b in banks:
                nc.tensor.matmul(b[:], zl[:], zr[:], start=True, stop=False)

            # hint only the engines whose loop body spans >1 IRAM block
            # (PE ~512 instrs, DVE ~450); ACT/Pool fit in one block and
            # would pay the per-edge hint_cnd write for nothing (measured
            # 76.0 -> 64.0 us/group when dropping them)
            if os.environ.get("KHINT", "pd") == "p":
                hints = (mybir.EngineType.PE,)
            else:
                hints = (mybir.EngineType.PE, mybir.EngineType.DVE)
            # staggered_reset measured ~4% faster (71.8 vs 76.0 us/group)
            # but one run died with NRT_EXEC_UNIT_UNRECOVERABLE; default to
            # the proven full-barrier back-edge for reliability.
            stag = os.environ.get("KSTAG", "0") == "1"
            with tc.For_i(0, loop_ng * 6 * GS, 6 * GS,
                          hint_engines=hints, staggered_reset=stag) as g0:
                st = stagep.tile([P, 6 * GS], F32)
                nc.sync.dma_start(st[:], fields.ap()[:, bass.ds(g0, 6 * GS)])
                ts_ = st[:, 0 * GS:1 * GS]
                x_ = st[:, 1 * GS:2 * GS]
                y_ = st[:, 2 * GS:3 * GS]
                pol_ = st[:, 3 * GS:4 * GS]
                fx_ = st[:, 4 * GS:5 * GS]
                fy_ = st[:, 5 * GS:6 * GS]

                # ---- per-group derived quantities [P, GS] (fp32) ----
                d = {k: drvp.tile([P, GS], F32, tag=k, name=k)
                     for k in ("upos", "uneg", "g1", "g2",
                               "wxf", "wxb", "nwxf", "nwxb", "nwyf", "nwyb")}
                # polarity masks (GPSIMD rejects TensorScalarPtr; these and
                # the stt ops below must stay on DVE)
                nc.vector.tensor_scalar(d["upos"][:], pol_, 0.0, None, OP.max)
                nc.vector.tensor_scalar(d["uneg"][:], pol_, -1.0, 0.0,
                                        OP.mult, OP.max)
                # warp positions: wx = x + (tref - ts) * fx * 256
                nc.gpsimd.tensor_mul(d["g1"][:], fx_, ts_)
                nc.gpsimd.tensor_mul(d["g2"][:], fy_, ts_)
                # bw (tref=0): wx_bw = x - 256*g1 ; fw: wx_fw = wx_bw + 256*fx
                nc.vector.scalar_tensor_tensor(d["wxb"][:], d["g1"][:], -FS,
                                               x_, OP.mult, OP.add)
                nc.vector.scalar_tensor_tensor(d["wxf"][:], fx_, FS,
                                               d["wxb"][:], OP.mult, OP.add)
                nc.vector.scalar_tensor_tensor(d["nwxb"][:], d["g1"][:], FS,
                                               x_, OP.mult, OP.subtract)
                nc.vector.scalar_tensor_tensor(d["nwxf"][:], fx_, -FS,
                                               d["nwxb"][:], OP.mult, OP.add)
                nc.vector.scalar_tensor_tensor(d["nwyb"][:], d["g2"][:], FS,
                                               y_, OP.mult, OP.subtract)
                nc.vector.scalar_tensor_tensor(d["nwyf"][:], fy_, -FS,
                                               d["nwyb"][:], OP.mult, OP.add)

                passes = ((0, d["wxf"], d["nwxf"], d["nwyf"]),
                          (1, d["wxb"], d["nwxb"], d["nwyb"]))
                for c in range(GS):
                    for (pi, wx, nwx, nwy) in passes:
                        # column indicators (DVE):
                        # t1 = relu(iota + 1 - wx); gxm = min(1 - iota + wx, t1)
                        t1x = ohp.tile([P, HW], F16, tag="t1x")
                        if kvar == "t1gps":
                            # relu deferred to the v0/v1 (max 0); plain add
                            # is GPSIMD-legal as broadcast tensor_tensor
                            nc.gpsimd.tensor_tensor(
                                t1x[:], iota_p1, _bcast(nwx[:, c:c + 1], HW),
                                OP.add)
                        else:
                            nc.vector.tensor_scalar(t1x[:], iota_p1,
                                                    nwx[:, c:c + 1], 0.0,
                                                    OP.add, OP.max)
                        gxm = ohp.tile([P, HW], F16, tag="gxm")
                        nc.vector.scalar_tensor_tensor(gxm[:], niota_p1,
                                                       wx[:, c:c + 1], t1x[:],
                                                       OP.add, OP.min)
                        # row indicators (ACT): gy = relu(1 - |iota - wy|)
                        absy = ohp.tile([P, HW], F16, tag="absy")
                        nc.scalar.activation(absy[:], iota_c, AF.Abs,
                                             bias=nwy[:, c:c + 1], scale=1.0)
                        gy = ohp.tile([P, HW], F16, tag="gy")
                        nc.scalar.activation(gy[:], absy[:], AF.Relu,
                                             bias=1.0, scale=-1.0)
                        # moving operand: polarity-masked relu(gxm) pair
                        rhs = rhsp.tile([P, 512], F16, tag="rhs")
                        nc.vector.tensor_scalar(rhs[:, 0:256], gxm[:], 0.0,
                                                d["upos"][:, c:c + 1],
                                                OP.max, OP.mult)
                        nc.vector.tensor_scalar(rhs[:, 256:512], gxm[:], 0.0,
                                                d["uneg"][:, c:c + 1],
                                                OP.max, OP.mult)
                        # ts weighting folded into a second stationary
                        # operand: gyts = gy * ts (broadcast tt on GPSIMD)
                        gyts = ohp.tile([P, HW], F16, tag="gyts")
                        nc.gpsimd.tensor_tensor(gyts[:], gy[:],
                                                _bcast(ts_[:, c:c + 1], HW),
                                                OP.mult)
                        for h in (0, 1):
                            nc.tensor.matmul(banks[pi * 4 + h * 2][:],
                                             gy[:, h * P:(h + 1) * P],
                                             rhs[:], start=False, stop=False)
                            nc.tensor.matmul(banks[pi * 4 + h * 2 + 1][:],
                                             gyts[:, h * P:(h + 1) * P],
                                             rhs[:], start=False, stop=False)

            # close accumulation groups
            for b in banks:
                nc.tensor.matmul(b[:], zl[:], zr[:], start=False, stop=True)
            # drain PSUM -> SBUF -> DRAM
            for i, b in enumerate(banks):
                ob = outp.tile([P, 512], F32, tag=f"ob{i}")
                if i % 2 == 0:
                    nc.vector.tensor_copy(ob[:], b[:])
                else:
                    nc.scalar.copy(ob[:], b[:])
                nc.sync.dma_start(hist.ap()[i], ob[:])

    nc.compile()
    return nc


def _iota_arrays():
    c = np.arange(HW, dtype=np.float32)
    rows = np.concatenate([c + 1.0, 1.0 - c, c]).astype(np.float16)
    return np.broadcast_to(rows, (P, 3 * HW)).copy()


def _pack_fields(ev, fl, ng=NG):
    """ev [n,4] fp32, fl [n,2] fp32 -> [P, ng*6*GS] fp32 (pol=0 padding)."""
    nch = ng * GS
    evc = nch * P
    n = ev.shape[0]
    arr = np.zeros((6, evc), dtype=np.float32)
    arr[0, :n] = ev[:, 0]
    arr[1, :n] = ev[:, 1]
    arr[2, :n] = ev[:, 2]
    arr[3, :n] = ev[:, 3]
    arr[4, :n] = fl[:, 0]
    arr[5, :n] = fl[:, 1]
    # [6, nch, P] -> [P, ng, 6, GS]
    a = arr.reshape(6, ng, GS, P)
    return np.ascontiguousarray(a.transpose(3, 1, 0, 2)).reshape(P, ng * 6 * GS)


_PROG = {}


def _get_prog():
    if "nc" not in _PROG:
        _PROG["nc"] = build_program(NG)
    return _PROG["nc"]


def loss_from_hists(hists):
    """hists: list of 2 arrays [8,128,512] (one per batch, summed over
    that batch's cores). Returns the scalar loss (float64)."""
    total = 0.0
    for hb in hists:
        for pi in range(2):
            iwe_p = np.empty((HW, HW), np.float64)
            iwe_n = np.empty((HW, HW), np.float64)
            ts_p = np.empty((HW, HW), np.float64)
            ts_n = np.empty((HW, HW), np.float64)
            for h in (0, 1):
                b0 = hb[pi * 4 + h * 2]       # [128, 512] = v0|v1
                b1 = hb[pi * 4 + h * 2 + 1]   # [128, 512] = v2|v3
                iwe_p[h * P:(h + 1) * P] = b0[:, 0:256]
                iwe_n[h * P:(h + 1) * P] = b0[:, 256:512]
                ts_p[h * P:(h + 1) * P] = b1[:, 0:256]
                ts_n[h * P:(h + 1) * P] = b1[:, 256:512]
            l = (ts_p / (iwe_p + EPS)) ** 2 + (ts_n / (iwe_n + EPS)) ** 2
            nz = ((iwe_p + iwe_n) > 0).sum()
            total += l.sum() / nz
    return total


def kernel(events, flow):
    global LAST_EXEC_NS, LAST_RESULTS
    events = np.asarray(events, dtype=np.float32)
    flow = np.asarray(flow, dtype=np.float32)
    B, N = events.shape[0], events.shape[1]
    assert B == 2 and N == CORES_PER_BATCH * EV_REAL, (B, N)

    nc = _get_prog()
    iotas = _iota_arrays()
    in_maps = []
    for core in range(NCORES):
        b, j = divmod(core, CORES_PER_BATCH)
        sl = slice(j * EV_REAL, (j + 1) * EV_REAL)
        in_maps.append({
            "fields": _pack_fields(events[b, sl], flow[b, sl]),
            "iotas": iotas,
        })

    res = run_bass_kernel_spmd(nc, in_maps, core_ids=list(range(NCORES)))
    LAST_RESULTS = res
    LAST_EXEC_NS = res.exec_time_ns

    hists = []
    for b in range(2):
        hb = np.zeros((8, P, 512), np.float64)
        for j in range(CORES_PER_BATCH):
            hb += res.results[b * CORES_PER_BATCH + j]["hist"]
        hists.append(hb)
    return np.float32(loss_from_hists(hists))

